# revision 1
# baseline (speedup 1.0000x reference)
"""Trainium2 Bass kernel for nn_CuteInferLinear (quantized linear, fp8-e4m3fn emulation).

Math (per reference):
  xq, xs = quantize(x, chunk=128)   per-row/per-128-col-group fp8_e4m3fn quant
  wq, ws = quantize(W, chunk=128)
  out = (xq*rep(xs)) @ (wq*rep(ws)).T + bias        -> bf16

Implementation notes:
  * TRN2's float8e4 is e4m3 with max +-240 (IEEE-ish), NOT OCP e4m3fn (max 448).
    Quantizing with scale amax/224 instead of amax/448 and dequantizing with the
    matching scale is bit-equivalent for all normal values (pure exponent shift),
    so we use 224.
  * Dequantized xd/wd are rounded to bf16 for the TensorEngine matmul (PSUM f32
    accumulate). End-to-end rel-l2 error vs the f32 reference ~3.6e-3.
  * Tensor-parallel over 8 cores: W/bias/out sharded on N, x replicated.
  * Per core: quantize-dequantize x and the W slice into DRAM scratch, then
    DMA-transpose-load (xbar) K-major tiles and run a resident-W blocked GEMM.
"""

import math

import numpy as np
import ml_dtypes

import concourse.bass as bass
import concourse.mybir as mybir
import concourse.tile as tile
from concourse import bacc

P = 128
FP8_SCALE = 224.0  # trn float8e4 max is 240; use 224 = 448/2 (exponent shift of ref's 448)
EPS = 1e-4

BF16 = mybir.dt.bfloat16
F32 = mybir.dt.float32
FP8 = mybir.dt.float8e4


def build_core_program(
    M: int,
    K: int,
    NL: int,
    MP: int = 512,       # m-panel rows per xdT load
    NBLK: int = 512,     # psum block (free dim per matmul)
    n_splits: int = 2,   # N halves; wdT for one half resident in SBUF at a time
    PREK: int = 2048,    # preproc K chunk
    num_devices: int = 8,
):
    """Builds the per-core Bass program. Every core runs this same program on
    its own shard: x [M,K] replicated, w [NL,K], bias [NL], out [M,NL]."""
    KO = K // P
    assert K % P == 0 and M % MP == 0 and MP % P == 0
    assert NL % n_splits == 0
    NH = NL // n_splits
    NBLK = min(NBLK, NH)
    assert NH % NBLK == 0
    NB = NH // NBLK
    MPT = MP // P
    PREK = min(PREK, K)
    assert K % PREK == 0 and PREK % P == 0
    PCH = K // PREK
    GC = PREK // P  # groups per preproc chunk

    nc = bacc.Bacc(
        "TRN2",
        target_bir_lowering=False,
        debug=False,
        enable_asserts=True,
        num_devices=num_devices,
    )

    x_in = nc.dram_tensor("x", [M, K], BF16, kind="ExternalInput").ap()
    w_in = nc.dram_tensor("w", [NL, K], BF16, kind="ExternalInput").ap()
    b_in = nc.dram_tensor("bias", [NL], BF16, kind="ExternalInput").ap()
    out = nc.dram_tensor("out", [M, NL], BF16, kind="ExternalOutput").ap()
    xd_dram = nc.dram_tensor("xd_scratch", [M, K], BF16).ap()
    wd_dram = nc.dram_tensor("wd_scratch", [NL, K], BF16).ap()

    with tile.TileContext(nc) as tc:
        with (
            tc.tile_pool(name="const", bufs=1) as const,
            tc.tile_pool(name="wdt", bufs=1) as wdt_pool,
            tc.tile_pool(name="xdt", bufs=2) as xdt_pool,
            tc.tile_pool(name="prep", bufs=2) as prep,
            tc.tile_pool(name="stat", bufs=3) as stat,
            tc.tile_pool(name="psum", bufs=6, space="PSUM") as psum_pool,
            tc.tile_pool(name="outp", bufs=3) as outp,
        ):
            # bias broadcast onto all 128 partitions
            bias_sb = const.tile([P, NL], BF16)
            bias_bcast = bass.AP(
                tensor=b_in.tensor,
                offset=b_in.offset,
                ap=[[0, P], b_in.ap[0]],
            )
            nc.gpsimd.dma_start(out=bias_sb[:], in_=bias_bcast)

            def quant_dequant_rows(src, dst, row0):
                """fp8 quantize+dequantize one [P, K] row-tile src->dst (DRAM)."""
                for c in range(PCH):
                    ks = bass.ts(c, PREK)
                    xt = prep.tile([P, PREK], BF16, tag="pt_in")
                    nc.sync.dma_start(out=xt[:], in_=src[row0 : row0 + P, ks])
                    amax = stat.tile([P, GC], F32, tag="amax")
                    nc.vector.tensor_reduce(
                        out=amax[:],
                        in_=xt.rearrange("p (g c) -> p g c", c=P),
                        axis=mybir.AxisListType.X,
                        op=mybir.AluOpType.max,
                        apply_absolute_value=True,
                    )
                    nc.vector.tensor_scalar_max(amax[:], amax[:], EPS)
                    inv = stat.tile([P, GC], F32, tag="inv")
                    nc.vector.reciprocal(out=inv[:], in_=amax[:])
                    nc.vector.tensor_scalar_mul(inv[:], inv[:], FP8_SCALE)
                    sc = stat.tile([P, GC], F32, tag="sc")
                    nc.vector.tensor_scalar_mul(sc[:], amax[:], 1.0 / FP8_SCALE)
                    qt = prep.tile([P, PREK], FP8, tag="pt_q")
                    dt_ = prep.tile([P, PREK], BF16, tag="pt_d")
                    for g in range(GC):
                        gs = bass.ts(g, P)
                        # quantize on ACT (per-partition scale operand)
                        nc.scalar.activation(
                            out=qt[:, gs],
                            in_=xt[:, gs],
                            func=mybir.ActivationFunctionType.Copy,
                            scale=inv[:, g : g + 1],
                        )
                        # dequantize: alternate DVE/ACT to balance engines
                        if g % 2 == 0:
                            nc.vector.tensor_scalar_mul(
                                dt_[:, gs], qt[:, gs], sc[:, g : g + 1]
                            )
                        else:
                            nc.scalar.activation(
                                out=dt_[:, gs],
                                in_=qt[:, gs],
                                func=mybir.ActivationFunctionType.Copy,
                                scale=sc[:, g : g + 1],
                            )
                    nc.sync.dma_start(out=dst[row0 : row0 + P, ks], in_=dt_[:])

            n_panels = M // MP
            # W-preproc row-tiles for halves >=1 get interleaved into half 0's
            # m-panel loop so the PE never waits on them at half boundaries.
            deferred_w_rows = [
                h * NH + t * P for h in range(1, n_splits) for t in range(NH // P)
            ]
            per_panel = (
                math.ceil(len(deferred_w_rows) / max(1, n_panels - 1))
                if deferred_w_rows
                else 0
            )

            for half in range(n_splits):
                nh0 = half * NH
                wdT = wdt_pool.tile([P, KO, NH], BF16, tag="wdT")
                if half == 0:
                    for t in range(NH // P):
                        quant_dequant_rows(w_in, wd_dram, nh0 + t * P)
                # transposed load of this half's wd into SBUF (K on partitions)
                for nbi in range(NB):
                    nrows = bass.ds(nh0 + nbi * NBLK, NBLK)
                    for ko in range(KO):
                        nc.sync.dma_start_transpose(
                            out=wdT[:, ko, bass.ts(nbi, NBLK)],
                            in_=wd_dram[nrows, bass.ts(ko, P)],
                        )

                for mp in range(n_panels):
                    mrow0 = mp * MP
                    if half == 0:
                        for t in range(MPT):
                            quant_dequant_rows(x_in, xd_dram, mrow0 + t * P)
                        # sprinkle later-half W preproc into panels 0..n-2
                        for _ in range(per_panel):
                            if deferred_w_rows:
                                quant_dequant_rows(
                                    w_in, wd_dram, deferred_w_rows.pop(0)
                                )
                    xdT = xdt_pool.tile([P, KO, MP], BF16, tag="xdT")
                    for ko in range(KO):
                        nc.sync.dma_start_transpose(
                            out=xdT[:, ko, :],
                            in_=xd_dram[mrow0 : mrow0 + MP, bass.ts(ko, P)],
                        )
                    for ms in range(MPT):
                        ot = outp.tile([P, NH], BF16, tag="osb")
                        for nbi in range(NB):
                            ps = psum_pool.tile([P, NBLK], F32, tag="ps")
                            for ko in range(KO):
                                nc.tensor.matmul(
                                    ps[:],
                                    xdT[:, ko, bass.ts(ms, P)],
                                    wdT[:, ko, bass.ts(nbi, NBLK)],
                                    start=(ko == 0),
                                    stop=(ko == KO - 1),
                                )
                            nc.vector.tensor_add(
                                ot[:, bass.ts(nbi, NBLK)],
                                ps[:],
                                bias_sb[:, bass.ds(nh0 + nbi * NBLK, NBLK)],
                            )
                        nc.sync.dma_start(
                            out=out[
                                mrow0 + ms * P : mrow0 + (ms + 1) * P,
                                bass.ds(nh0, NH),
                            ],
                            in_=ot[:],
                        )

    nc.compile()
    return nc


_CACHE = {}


def _get_program(M, K, NL, **kw):
    key = (M, K, NL, tuple(sorted(kw.items())))
    if key not in _CACHE:
        _CACHE[key] = build_core_program(M, K, NL, **kw)
    return _CACHE[key]


def kernel(x, W, bias, chunk_size=128, int8=0, **_unused):
    """Full-input entry: shards across 8 NeuronCores (column-parallel) and
    returns the full [M, N] output."""
    from concourse.bass_utils import run_bass_kernel_spmd

    assert int(chunk_size) == 128 and int(int8) == 0
    x = np.asarray(x)
    W = np.asarray(W)
    bias = np.asarray(bias)
    M, K = x.shape
    N = W.shape[0]
    n_cores = 8
    assert N % n_cores == 0
    NL = N // n_cores

    nc = _get_program(M, K, NL)

    bf = ml_dtypes.bfloat16
    xb = np.ascontiguousarray(x.astype(bf, copy=False))
    in_maps = []
    for i in range(n_cores):
        in_maps.append(
            {
                "x": xb,
                "w": np.ascontiguousarray(W[i * NL : (i + 1) * NL].astype(bf, copy=False)),
                "bias": np.ascontiguousarray(bias[i * NL : (i + 1) * NL].astype(bf, copy=False)),
            }
        )

    res = run_bass_kernel_spmd(nc, in_maps, core_ids=list(range(n_cores)))
    outs = [res.results[i]["out"] for i in range(n_cores)]
    full = np.concatenate(outs, axis=1)
    return full.astype(x.dtype, copy=False)


# revision 5
# speedup vs baseline: 1.0175x; 1.0175x over previous
"""Trainium2 Bass kernel for nn_CuteInferLinear (quantized linear, fp8-e4m3fn emulation).

Math (per reference):
  xq, xs = quantize(x, chunk=128)   per-row/per-128-col-group fp8_e4m3fn quant
  wq, ws = quantize(W, chunk=128)
  out = (xq*rep(xs)) @ (wq*rep(ws)).T + bias        -> bf16

Implementation notes:
  * TRN2's float8e4 is e4m3 with max +-240 (IEEE-ish), NOT OCP e4m3fn (max 448).
    Quantizing with scale amax/224 instead of amax/448 and dequantizing with the
    matching scale is bit-equivalent for all normal values (pure exponent shift),
    so we use 224.
  * Dequantized xd/wd are rounded to bf16 for the TensorEngine matmul (PSUM f32
    accumulate). End-to-end rel-l2 error vs the f32 reference ~3.6e-3.
  * Tensor-parallel over 8 cores: W/bias/out sharded on N, x replicated.
  * Per core: quantize-dequantize x and the W slice into DRAM scratch using
    broadcast-AP (0-stride) tensor_tensor ops (quant on DVE, dequant on GPSIMD)
    so per-group scales cost one instruction per K-chunk, then DMA-transpose
    (xbar) K-major tiles and run a fully-resident-W blocked GEMM on the PE.
  * DMA issue is split between the two HWDGE sequencers (SP + ACT).
"""

import numpy as np
import ml_dtypes

import concourse.bass as bass
import concourse.mybir as mybir
import concourse.tile as tile
from concourse import bacc

P = 128
FP8_SCALE = 224.0  # trn float8e4 max is 240; 224 = 448/2 (exponent shift of ref's 448)
EPS = 1e-4
SPLIT_DMA_ISSUE = False  # split DMA issue across SP+ACT HWDGE rings
GPSIMD_DEQUANT = True    # run the dequant pass on GPSIMD (else DVE)

BF16 = mybir.dt.bfloat16
F32 = mybir.dt.float32
FP8 = mybir.dt.float8e4


def _bcast(stat_ap, g0, ng, c):
    """View stat[:, g0:g0+ng] as [P, ng, c] with 0-stride inner dim."""
    base = stat_ap[:, g0 : g0 + ng]
    return bass.AP(tensor=base.tensor, offset=base.offset, ap=[base.ap[0], base.ap[1], [0, c]])


def build_core_program(
    M: int,
    K: int,
    NL: int,
    MP: int = 256,       # m-panel rows per xdT load
    NBLK: int = 512,     # psum block (free dim per matmul)
    PREK: int = 1024,    # preproc K chunk
    num_devices: int = 8,
):
    KO = K // P
    assert K % P == 0 and M % MP == 0 and MP % P == 0
    NBLK = min(NBLK, NL)
    assert NL % NBLK == 0
    NB = NL // NBLK
    MPT = MP // P
    PREK = min(PREK, K)
    assert K % PREK == 0 and PREK % P == 0
    PCH = K // PREK
    GC = PREK // P  # groups per preproc chunk

    nc = bacc.Bacc(
        "TRN2",
        target_bir_lowering=False,
        debug=False,
        enable_asserts=True,
        num_devices=num_devices,
    )

    x_in = nc.dram_tensor("x", [M, K], BF16, kind="ExternalInput").ap()
    w_in = nc.dram_tensor("w", [NL, K], BF16, kind="ExternalInput").ap()
    b_in = nc.dram_tensor("bias", [NL], BF16, kind="ExternalInput").ap()
    out = nc.dram_tensor("out", [M, NL], BF16, kind="ExternalOutput").ap()
    xd_dram = nc.dram_tensor("xd_scratch", [M, K], BF16).ap()
    wd_dram = nc.dram_tensor("wd_scratch", [NL, K], BF16).ap()

    # alternate the two HWDGE issuers for DMA instructions
    _dma_flip = [0]

    def dma_eng():
        if not SPLIT_DMA_ISSUE:
            return nc.sync
        _dma_flip[0] ^= 1
        return nc.sync if _dma_flip[0] else nc.scalar

    with tile.TileContext(nc) as tc:
        with (
            tc.tile_pool(name="const", bufs=1) as const,
            tc.tile_pool(name="wdt", bufs=1) as wdt_pool,
            tc.tile_pool(name="xdt", bufs=2) as xdt_pool,
            tc.tile_pool(name="prepx", bufs=PCH + 1) as prepx,
            tc.tile_pool(name="prepqd", bufs=2) as prepqd,
            tc.tile_pool(name="stat", bufs=3) as stat,
            tc.tile_pool(name="psum", bufs=6, space="PSUM") as psum_pool,
            tc.tile_pool(name="outp", bufs=4) as outp,
        ):
            # bias broadcast onto all 128 partitions
            bias_sb = const.tile([P, NL], BF16)
            bias_bcast = bass.AP(
                tensor=b_in.tensor,
                offset=b_in.offset,
                ap=[[0, P], b_in.ap[0]],
            )
            nc.gpsimd.dma_start(out=bias_sb[:], in_=bias_bcast)

            def quant_dequant_rows(src, dst, row0):
                """fp8 quantize+dequantize one [P, K] row-tile src->dst (DRAM).
                Stats once per row-tile; per-K-chunk broadcast ops:
                DVE quant (bf16*inv -> fp8), GPSIMD dequant (fp8*sc -> bf16)."""
                xts = []
                amax = stat.tile([P, KO], F32, tag="amax")
                for c in range(PCH):
                    xt = prepx.tile([P, PREK], BF16, tag="pt_in")
                    dma_eng().dma_start(out=xt[:], in_=src[row0 : row0 + P, bass.ts(c, PREK)])
                    nc.vector.tensor_reduce(
                        out=amax[:, c * GC : (c + 1) * GC],
                        in_=xt.rearrange("p (g c) -> p g c", c=P),
                        axis=mybir.AxisListType.X,
                        op=mybir.AluOpType.max,
                        apply_absolute_value=True,
                    )
                    xts.append(xt)
                nc.vector.tensor_scalar_max(amax[:], amax[:], EPS)
                inv = stat.tile([P, KO], F32, tag="inv")
                nc.vector.reciprocal(out=inv[:], in_=amax[:])
                nc.vector.tensor_scalar_mul(inv[:], inv[:], FP8_SCALE)
                sc = stat.tile([P, KO], F32, tag="sc")
                nc.vector.tensor_scalar_mul(sc[:], amax[:], 1.0 / FP8_SCALE)
                for c in range(PCH):
                    xt = xts[c]
                    qt = prepqd.tile([P, PREK], FP8, tag="pt_q")
                    dt_ = prepqd.tile([P, PREK], BF16, tag="pt_d")
                    nc.vector.tensor_tensor(
                        out=qt.rearrange("p (g c) -> p g c", c=P),
                        in0=xt.rearrange("p (g c) -> p g c", c=P),
                        in1=_bcast(inv, c * GC, GC, P),
                        op=mybir.AluOpType.mult,
                    )
                    deq_eng = nc.gpsimd if GPSIMD_DEQUANT else nc.vector
                    deq_eng.tensor_tensor(
                        out=dt_.rearrange("p (g c) -> p g c", c=P),
                        in0=qt.rearrange("p (g c) -> p g c", c=P),
                        in1=_bcast(sc, c * GC, GC, P),
                        op=mybir.AluOpType.mult,
                    )
                    dma_eng().dma_start(out=dst[row0 : row0 + P, bass.ts(c, PREK)], in_=dt_[:])

            n_panels = M // MP

            # x panel 0 preproc first (its result gates the very first matmul)
            for t in range(MPT):
                quant_dequant_rows(x_in, xd_dram, t * P)
            # all W preproc upfront (wdT must be fully resident before the
            # first msub finishes its nb sweep anyway)
            for t in range(NL // P):
                quant_dequant_rows(w_in, wd_dram, t * P)

            # transposed load of wd into SBUF, K on partitions (xbar DMA)
            wdT = wdt_pool.tile([P, KO, NL], BF16)
            for nbi in range(NB):
                for ko in range(KO):
                    dma_eng().dma_start_transpose(
                        out=wdT[:, ko, bass.ts(nbi, NBLK)],
                        in_=wd_dram[bass.ds(nbi * NBLK, NBLK), bass.ts(ko, P)],
                    )

            for mp in range(n_panels):
                mrow0 = mp * MP
                xdT = xdt_pool.tile([P, KO, MP], BF16, tag="xdT")
                for ko in range(KO):
                    dma_eng().dma_start_transpose(
                        out=xdT[:, ko, :],
                        in_=xd_dram[mrow0 : mrow0 + MP, bass.ts(ko, P)],
                    )
                # JIT preproc of the NEXT panel (runs ahead on DVE/GPSIMD)
                if mp + 1 < n_panels:
                    for t in range(MPT):
                        quant_dequant_rows(x_in, xd_dram, (mp + 1) * MP + t * P)
                for ms in range(MPT):
                    for nbi in range(NB):
                        ps = psum_pool.tile([P, NBLK], F32, tag="ps")
                        for ko in range(KO):
                            nc.tensor.matmul(
                                ps[:],
                                xdT[:, ko, bass.ts(ms, P)],
                                wdT[:, ko, bass.ts(nbi, NBLK)],
                                start=(ko == 0),
                                stop=(ko == KO - 1),
                            )
                        ot = outp.tile([P, NBLK], BF16, tag="osb")
                        nc.vector.tensor_add(
                            ot[:],
                            ps[:],
                            bias_sb[:, bass.ts(nbi, NBLK)],
                        )
                        dma_eng().dma_start(
                            out=out[
                                mrow0 + ms * P : mrow0 + (ms + 1) * P,
                                bass.ts(nbi, NBLK),
                            ],
                            in_=ot[:],
                        )

    nc.compile()
    return nc


_CACHE = {}


def _get_program(M, K, NL, **kw):
    key = (M, K, NL, tuple(sorted(kw.items())))
    if key not in _CACHE:
        _CACHE[key] = build_core_program(M, K, NL, **kw)
    return _CACHE[key]


def kernel(x, W, bias, chunk_size=128, int8=0, **_unused):
    """Full-input entry: shards across 8 NeuronCores (column-parallel) and
    returns the full [M, N] output."""
    from concourse.bass_utils import run_bass_kernel_spmd

    assert int(chunk_size) == 128 and int(int8) == 0
    x = np.asarray(x)
    W = np.asarray(W)
    bias = np.asarray(bias)
    M, K = x.shape
    N = W.shape[0]
    n_cores = 8
    assert N % n_cores == 0
    NL = N // n_cores

    nc = _get_program(M, K, NL)

    bf = ml_dtypes.bfloat16
    xb = np.ascontiguousarray(x.astype(bf, copy=False))
    in_maps = []
    for i in range(n_cores):
        in_maps.append(
            {
                "x": xb,
                "w": np.ascontiguousarray(W[i * NL : (i + 1) * NL].astype(bf, copy=False)),
                "bias": np.ascontiguousarray(bias[i * NL : (i + 1) * NL].astype(bf, copy=False)),
            }
        )

    res = run_bass_kernel_spmd(nc, in_maps, core_ids=list(range(n_cores)))
    outs = [res.results[i]["out"] for i in range(n_cores)]
    full = np.concatenate(outs, axis=1)
    return full.astype(x.dtype, copy=False)


# revision 7
# speedup vs baseline: 1.1170x; 1.0977x over previous
"""Trainium2 Bass kernel for nn_CuteInferLinear (quantized linear, fp8-e4m3fn emulation).

Math (per reference):
  xq, xs = quantize(x, chunk=128)   per-row/per-128-col-group fp8_e4m3fn quant
  wq, ws = quantize(W, chunk=128)
  out = (xq*rep(xs)) @ (wq*rep(ws)).T + bias        -> bf16

Implementation notes:
  * TRN2's float8e4 is e4m3 with max +-240, NOT OCP e4m3fn (max 448). Quantizing
    with scale amax/224 instead of amax/448 (and dequantizing to match) is
    bit-equivalent for normals (pure exponent shift), so we use 224.
  * Dequantized xd/wd are rounded to bf16 for the TensorEngine matmul (PSUM f32
    accumulate). End-to-end rel-l2 error vs the f32 reference ~3.6e-3.
  * Tensor-parallel over 8 cores: W/bias/out sharded on N, x replicated.
  * Preproc: broadcast-AP (0-stride) tensor_tensor ops -- quant on DVE,
    dequant on GPSIMD -- one instruction per K-chunk.
  * DMA_TRANSPOSE issue cost on the sequencer is ~1.25us per instruction
    regardless of size, so transposes are batched big: xd goes x->xd_dram
    (natural) -> [1024,128] xbar transposes -> xdT_dram (K-major), and matmul
    panels load with ONE plain DMA each. wd transposes straight into the
    SBUF-resident wdT.
  * bias is added via a K=1 matmul row that opens each PSUM accumulation
    group (start=True), so eviction is a pure ACT copy (DVE stays free).
  * All HWDGE DMA issue stays on nc.sync: splitting across the SP+ACT rings
    corrupts xbar transposes at scale (verified empirically).
"""

import numpy as np
import ml_dtypes

import concourse.bass as bass
import concourse.mybir as mybir
import concourse.tile as tile
from concourse import bacc

P = 128
FP8_SCALE = 224.0
EPS = 1e-4

BF16 = mybir.dt.bfloat16
F32 = mybir.dt.float32
FP8 = mybir.dt.float8e4


def _bcast(stat_ap, g0, ng, c):
    """View stat[:, g0:g0+ng] as [P, ng, c] with 0-stride inner dim."""
    base = stat_ap[:, g0 : g0 + ng]
    return bass.AP(tensor=base.tensor, offset=base.offset, ap=[base.ap[0], base.ap[1], [0, c]])


def build_core_program(
    M: int,
    K: int,
    NL: int,
    MP: int = 256,       # m-panel rows per xdT SBUF load
    NBLK: int = 512,     # psum block (free dim per matmul)
    PREK: int = 2048,    # preproc K chunk
    XBLK: int = 1024,    # x transpose block rows
    WSPL: int = 2,       # W transpose gating splits (rows NL/WSPL per xbar DMA)
    num_devices: int = 8,
):
    KO = K // P
    assert K % P == 0 and M % MP == 0 and MP % P == 0 and M % XBLK == 0 and XBLK % MP == 0
    NBLK = min(NBLK, NL)
    assert NL % NBLK == 0
    NB = NL // NBLK
    MPT = MP // P
    PREK = min(PREK, K)
    assert K % PREK == 0 and PREK % P == 0
    PCH = K // PREK
    GC = PREK // P
    assert NL % WSPL == 0
    NW = NL // WSPL

    nc = bacc.Bacc(
        "TRN2",
        target_bir_lowering=False,
        debug=False,
        enable_asserts=True,
        num_devices=num_devices,
    )

    x_in = nc.dram_tensor("x", [M, K], BF16, kind="ExternalInput").ap()
    w_in = nc.dram_tensor("w", [NL, K], BF16, kind="ExternalInput").ap()
    b_in = nc.dram_tensor("bias", [NL], BF16, kind="ExternalInput").ap()
    out = nc.dram_tensor("out", [M, NL], BF16, kind="ExternalOutput").ap()
    xd_dram = nc.dram_tensor("xd_scratch", [M, K], BF16).ap()
    wd_dram = nc.dram_tensor("wd_scratch", [NL, K], BF16).ap()
    xdt_dram = nc.dram_tensor("xdt_scratch", [KO, P, M], BF16).ap()

    with tile.TileContext(nc) as tc:
        with (
            tc.tile_pool(name="const", bufs=1) as const,
            tc.tile_pool(name="wdt", bufs=1) as wdt_pool,
            tc.tile_pool(name="xdt", bufs=2) as xdt_pool,
            tc.tile_pool(name="stg", bufs=2) as stg_pool,
            tc.tile_pool(name="prepx", bufs=PCH + 1) as prepx,
            tc.tile_pool(name="prepqd", bufs=2) as prepqd,
            tc.tile_pool(name="stat", bufs=3) as stat,
            tc.tile_pool(name="psum", bufs=6, space="PSUM") as psum_pool,
            tc.tile_pool(name="outp", bufs=2) as outp,
        ):
            # bias row (partition 0) + ones column for the K=1 bias matmul
            bias_sb = const.tile([P, NL], BF16)
            nc.sync.dma_start(out=bias_sb[0:1, :], in_=b_in[None, :])
            ones_sb = const.tile([P, P], BF16)
            nc.vector.memset(ones_sb[:], 1.0)

            def quant_dequant_rows(src, dst, row0):
                """fp8 quantize+dequantize one [P, K] row-tile src->dst (DRAM)."""
                xts = []
                amax = stat.tile([P, KO], F32, tag="amax")
                for c in range(PCH):
                    xt = prepx.tile([P, PREK], BF16, tag="pt_in")
                    nc.sync.dma_start(out=xt[:], in_=src[row0 : row0 + P, bass.ts(c, PREK)])
                    nc.vector.tensor_reduce(
                        out=amax[:, c * GC : (c + 1) * GC],
                        in_=xt.rearrange("p (g c) -> p g c", c=P),
                        axis=mybir.AxisListType.X,
                        op=mybir.AluOpType.max,
                        apply_absolute_value=True,
                    )
                    xts.append(xt)
                nc.vector.tensor_scalar_max(amax[:], amax[:], EPS)
                inv = stat.tile([P, KO], F32, tag="inv")
                nc.vector.reciprocal(out=inv[:], in_=amax[:])
                nc.vector.tensor_scalar_mul(inv[:], inv[:], FP8_SCALE)
                sc = stat.tile([P, KO], F32, tag="sc")
                nc.vector.tensor_scalar_mul(sc[:], amax[:], 1.0 / FP8_SCALE)
                for c in range(PCH):
                    xt = xts[c]
                    qt = prepqd.tile([P, PREK], FP8, tag="pt_q")
                    dt_ = prepqd.tile([P, PREK], BF16, tag="pt_d")
                    nc.vector.tensor_tensor(
                        out=qt.rearrange("p (g c) -> p g c", c=P),
                        in0=xt.rearrange("p (g c) -> p g c", c=P),
                        in1=_bcast(inv, c * GC, GC, P),
                        op=mybir.AluOpType.mult,
                    )
                    nc.gpsimd.tensor_tensor(
                        out=dt_.rearrange("p (g c) -> p g c", c=P),
                        in0=qt.rearrange("p (g c) -> p g c", c=P),
                        in1=_bcast(sc, c * GC, GC, P),
                        op=mybir.AluOpType.mult,
                    )
                    nc.sync.dma_start(out=dst[row0 : row0 + P, bass.ts(c, PREK)], in_=dt_[:])

            def x_block_transpose(blk):
                """xd_dram rows [blk*XBLK, +XBLK) -> xdT_dram [ko, p, mrange]."""
                m0 = blk * XBLK
                for ko in range(KO):
                    stg = stg_pool.tile([P, XBLK], BF16, tag="stg")
                    nc.sync.dma_start_transpose(
                        out=stg[:],
                        in_=xd_dram[m0 : m0 + XBLK, bass.ts(ko, P)],
                    )
                    nc.sync.dma_start(out=xdt_dram[ko, :, m0 : m0 + XBLK], in_=stg[:])

            n_xblk = M // XBLK

            # x block 0 preproc first, then all W preproc
            for t in range(XBLK // P):
                quant_dequant_rows(x_in, xd_dram, t * P)
            for t in range(NL // P):
                quant_dequant_rows(w_in, wd_dram, t * P)

            # wd -> wdT (SBUF resident, K on partitions); gated per WSPL rows
            wdT = wdt_pool.tile([P, KO, NL], BF16)
            for ws in range(WSPL):
                for ko in range(KO):
                    nc.sync.dma_start_transpose(
                        out=wdT[:, ko, bass.ts(ws, NW)],
                        in_=wd_dram[bass.ds(ws * NW, NW), bass.ts(ko, P)],
                    )

            x_block_transpose(0)

            panels_per_blk = XBLK // MP
            for mp in range(M // MP):
                blk = mp // panels_per_blk
                if mp % panels_per_blk == 0:
                    # JIT: preproc + transpose the NEXT x block while this one runs
                    if blk + 1 < n_xblk:
                        for t in range(XBLK // P):
                            quant_dequant_rows(x_in, xd_dram, (blk + 1) * XBLK + t * P)
                        x_block_transpose(blk + 1)
                mrow0 = mp * MP
                xdT = xdt_pool.tile([P, KO, MP], BF16, tag="xdT")
                nc.sync.dma_start(
                    out=xdT[:],
                    in_=xdt_dram.rearrange("ko p m -> p ko m")[:, :, mrow0 : mrow0 + MP],
                )
                for ms in range(MPT):
                    ot = outp.tile([P, NL], BF16, tag="osb")
                    psums = [
                        psum_pool.tile([P, NBLK], F32, tag="ps", name=f"ps{i}")
                        for i in range(NB)
                    ]
                    for nbi in range(NB):
                        # bias row opens the accumulation group (K=1 matmul)
                        nc.tensor.matmul(
                            psums[nbi][:],
                            ones_sb[0:1, :],
                            bias_sb[0:1, bass.ts(nbi, NBLK)],
                            start=True,
                            stop=False,
                        )
                    for ko in range(KO):
                        for nbi in range(NB):
                            nc.tensor.matmul(
                                psums[nbi][:],
                                xdT[:, ko, bass.ts(ms, P)],
                                wdT[:, ko, bass.ts(nbi, NBLK)],
                                start=False,
                                stop=(ko == KO - 1),
                            )
                    for nbi in range(NB):
                        nc.scalar.copy(out=ot[:, bass.ts(nbi, NBLK)], in_=psums[nbi][:])
                    nc.sync.dma_start(
                        out=out[mrow0 + ms * P : mrow0 + (ms + 1) * P, :],
                        in_=ot[:],
                    )

    nc.compile()
    return nc


_CACHE = {}


def _get_program(M, K, NL, **kw):
    key = (M, K, NL, tuple(sorted(kw.items())))
    if key not in _CACHE:
        _CACHE[key] = build_core_program(M, K, NL, **kw)
    return _CACHE[key]


def kernel(x, W, bias, chunk_size=128, int8=0, **_unused):
    """Full-input entry: shards across 8 NeuronCores (column-parallel) and
    returns the full [M, N] output."""
    from concourse.bass_utils import run_bass_kernel_spmd

    assert int(chunk_size) == 128 and int(int8) == 0
    x = np.asarray(x)
    W = np.asarray(W)
    bias = np.asarray(bias)
    M, K = x.shape
    N = W.shape[0]
    n_cores = 8
    assert N % n_cores == 0
    NL = N // n_cores

    nc = _get_program(M, K, NL)

    bf = ml_dtypes.bfloat16
    xb = np.ascontiguousarray(x.astype(bf, copy=False))
    in_maps = []
    for i in range(n_cores):
        in_maps.append(
            {
                "x": xb,
                "w": np.ascontiguousarray(W[i * NL : (i + 1) * NL].astype(bf, copy=False)),
                "bias": np.ascontiguousarray(bias[i * NL : (i + 1) * NL].astype(bf, copy=False)),
            }
        )

    res = run_bass_kernel_spmd(nc, in_maps, core_ids=list(range(n_cores)))
    outs = [res.results[i]["out"] for i in range(n_cores)]
    full = np.concatenate(outs, axis=1)
    return full.astype(x.dtype, copy=False)
